# revision 23
# baseline (speedup 1.0000x reference)
"""MultiHeadGAT Trainium2 kernel: 8-core batch-parallel, transposed-layout pipeline.

Math: for scores e = lrelu(s_i[n] + s_j[m]), softmax numerator
  p = exp(lrelu(s_i+s_j)) = e^{0.2 s_i} * max(e^{0.8 s_i} * e^{s_j}, e^{0.2 s_j})
The e^{0.2 s_i} row factor cancels in softmax, so on-device we only compute
  q[m, n] = adjT[m, n] * max(Wbc[m, n] * u[m], v[m])
with Wbc = broadcast(e^{0.8 s_i}) (n-varying), u = e^{s_j}, v = e^{0.2 s_j}
(per-partition scalars), which is one fused tensor_scalar (mult+max) plus one
tensor_tensor (mask) per tile. Attention output and row-sum Z come from one
PE matmul with lhsT = [ones | pad | Wh_head]; normalization 1/Z = exp(-ln(Z)).
"""

import sys

sys.path.insert(0, "/opt/trn_rl_repo")

import numpy as np

B, N, IN_DIM, H, HD = 8, 1024, 128, 8, 16
OUT_DIM = H * HD
EPS = 1e-5
NB = N // 128  # 8 m-blocks

_CACHE = {}


def _patch_act_tables():
    # Force one activation table set for the whole kernel: every function we
    # use (Exp, Ln, Copy, Square, Relu, Identity) lives in
    # natural_log_exp_and_others; emptying the other sets makes Bacc's
    # table-load inserter emit exactly one ACT_TABLE_LOAD instead of
    # thrashing between exp/ln/small sets (~2.5us per reload).
    import concourse.bacc as bacc
    import concourse.hw_specs as hw_specs
    if getattr(bacc, "_act_tables_patched", False):
        return
    orig = hw_specs.get_activation_tables

    def patched(arch):
        t = dict(orig(arch))
        keep = "natural_log_exp_and_others"
        return {k: (v if k == keep else set()) for k, v in t.items()}

    bacc.get_activation_tables = patched
    bacc._act_tables_patched = True


def _build_program():
    import concourse.bacc as bacc
    import concourse.mybir as mybir
    import concourse.tile as tile

    _patch_act_tables()

    F16 = mybir.dt.float16
    F32 = mybir.dt.float32
    AF = mybir.ActivationFunctionType
    OP = mybir.AluOpType

    nc = bacc.Bacc("TRN2", target_bir_lowering=False, debug=False, num_devices=8)

    # ---- I/O ----
    hT = nc.dram_tensor("hT", [128, N], F16, kind="ExternalInput")
    adjT = nc.dram_tensor("adjT", [128, NB * N], F16, kind="ExternalInput")
    # critical pack: [wcat 128 | wadst 8 | wasrep 1024]
    wpackA = nc.dram_tensor("wpackA", [128, 1160], F16, kind="ExternalInput")
    # late pack: [w1 256 | w2 256]
    wpackB = nc.dram_tensor("wpackB", [128, 512], F16, kind="ExternalInput")
    augpk = nc.dram_tensor("augpk", [128, 3072], F16, kind="ExternalInput")
    # packed f32 cols: [b1c 2 | b2c 1 | g1 1 | b1l 1 | g2 1 | b2l 1 | zbias 1 | eps 1]
    wpack32 = nc.dram_tensor("wpack32", [128, 9], F32, kind="ExternalInput")
    sel = nc.dram_tensor("sel", [16, H * 128], F16, kind="ExternalInput")
    e16cat = nc.dram_tensor("e16cat", [1, H * 128], F16, kind="ExternalInput")
    outT = nc.dram_tensor("outT", [128, N], F16, kind="ExternalOutput")

    with tile.TileContext(nc) as tc:
        with (
            tc.tile_pool(name="const", bufs=1) as cpool,
            tc.tile_pool(name="big", bufs=1) as big,
            tc.tile_pool(name="work", bufs=2) as work,
            tc.tile_pool(name="mid", bufs=1) as mid,
            tc.tile_pool(name="rows", bufs=1) as rows,
        ):
            # ---- load everything ----
            # All on the SP (sync) HWDGE ring: FIFO order = priority order.
            hT_t = cpool.tile([128, N], F16)
            nc.sync.dma_start(hT_t[:], hT[:])
            wpA = cpool.tile([128, 1160], F16)
            nc.sync.dma_start(wpA[:], wpackA[:])
            adjq = [
                cpool.tile([128, 4 * N], F16, tag=f"adj{i}", name=f"adj{i}")
                for i in range(2)
            ]
            nc.sync.dma_start(adjq[0][:], adjT[:, 0:4 * N])
            augt = cpool.tile([128, 3072], F16)
            nc.sync.dma_start(augt[:], augpk[:])
            nc.sync.dma_start(adjq[1][:], adjT[:, 4 * N:8 * N])
            wpB = cpool.tile([128, 512], F16)
            nc.sync.dma_start(wpB[:], wpackB[:])
            wp32 = cpool.tile([128, 9], F32)
            nc.sync.dma_start(wp32[:], wpack32[:])
            sel_t = cpool.tile([16, H * 128], F16)
            nc.sync.dma_start(sel_t[:], sel[:])
            e16cat_t = cpool.tile([1, H * 128], F16)
            nc.sync.dma_start(e16cat_t[:], e16cat[:])

            wcat_t = wpA[:, 0:128]
            wadst_t = wpA[:, 128:136]
            wasrep_t = wpA[:, 136:1160]
            w1_t = wpB[:, 0:256]
            w2_t = wpB[:, 256:512]
            aug = augt[:]
            b1_t = wp32[:, 0:2]
            b2_t = wp32[:, 2:3]
            g1_t = wp32[:, 3:4]
            b1l_t = wp32[:, 4:5]
            g2_t = wp32[:, 5:6]
            b2l_t = wp32[:, 6:7]
            zbias = wp32[:, 7:8]
            epsbias = wp32[:, 8:9]

            onescol = cpool.tile([128, 1], F16)
            nc.vector.memset(onescol[:], 1.0)
            jmat = cpool.tile([128, 128], F16)
            nc.vector.memset(jmat[:], 1.0 / 128)
            onesrow = cpool.tile([1, 128], F32)
            nc.vector.memset(onesrow[:], 1.0)

            # ---- phase 1: s-cols(u,v), Wbc, Wh_nat->aug ----
            u_t = [big.tile([128, H], F32, tag=f"u{i}", name=f"u{i}") for i in range(NB)]
            v_t = [big.tile([128, H], F32, tag=f"v{i}", name=f"v{i}") for i in range(NB)]
            wbc = [big.tile([128, N], F16, tag=f"wbc{i}", name=f"wbc{i}") for i in range(H)]
            aug4w = aug.rearrange("p (m h c) -> p m h c", m=NB, h=H, c=48)

            with tc.tile_pool(name="ps1", bufs=3, space="PSUM") as ps1:
                for mb in range(NB):
                    sc_ps = ps1.tile([128, H], F32, tag="ps1")
                    nc.tensor.matmul(
                        sc_ps[:], hT_t[:, mb * 128:(mb + 1) * 128], wadst_t,
                        start=True, stop=True,
                    )
                    nc.scalar.activation(u_t[mb][:], sc_ps[:], AF.Exp, scale=1.0)
                    nc.scalar.activation(v_t[mb][:], sc_ps[:], AF.Exp, scale=0.2)
                    if mb == 0:
                        wb_ps = ps1.tile([128, N], F32, tag="ps1")
                        for ch in range(2):
                            nc.tensor.matmul(
                                wb_ps[:, ch * 512:(ch + 1) * 512],
                                wasrep_t[:, 0:128],
                                hT_t[:, ch * 512:(ch + 1) * 512],
                                start=True, stop=True,
                            )
                        nc.scalar.activation(wbc[0][:], wb_ps[:], AF.Exp, scale=0.8)
                for hh in range(1, H):
                    wb_ps = ps1.tile([128, N], F32, tag="ps1")
                    for ch in range(2):
                        nc.tensor.matmul(
                            wb_ps[:, ch * 512:(ch + 1) * 512],
                            wasrep_t[:, hh * 128:(hh + 1) * 128],
                            hT_t[:, ch * 512:(ch + 1) * 512],
                            start=True, stop=True,
                        )
                    nc.scalar.activation(wbc[hh][:], wb_ps[:], AF.Exp, scale=0.8)
                for mb in range(NB):
                    wn_ps = ps1.tile([128, 128], F32, tag="ps1")
                    nc.tensor.matmul(
                        wn_ps[:], hT_t[:, mb * 128:(mb + 1) * 128], wcat_t,
                        start=True, stop=True,
                    )
                    wn4 = wn_ps[:].rearrange("p (h d) -> p h d", h=H, d=16)
                    nc.scalar.activation(aug4w[:, mb, :, 32:48], wn4[:], AF.Copy)

            # ---- phase 2: attention ----
            stage_all = big.tile([16, H * N], F16)
            with (
                tc.tile_pool(name="ps48", bufs=2, space="PSUM") as ps48,
                tc.tile_pool(name="psacc", bufs=1, space="PSUM") as psacc,
                tc.tile_pool(name="psz", bufs=1, space="PSUM") as psz,
            ):
                asm_ps = psacc.tile([128, N], F32)
                zbc_ps = psz.tile([128, N], F32)
                for hh in range(H):
                    q_all = work.tile([128, NB * N], F16, tag="q")
                    t1 = work.tile([128, NB * N], F16, tag="t1")
                    at_ps = ps48.tile([48, N], F32)
                    for half in range(2):
                        for mb in range(half * 4, half * 4 + 4):
                            nc.vector.tensor_scalar(
                                t1[:, mb * N:(mb + 1) * N],
                                wbc[hh][:],
                                u_t[mb][:, hh: hh + 1],
                                v_t[mb][:, hh: hh + 1],
                                op0=OP.mult, op1=OP.max,
                            )
                        nc.vector.tensor_tensor(
                            q_all[:, half * 4 * N:(half + 1) * 4 * N],
                            t1[:, half * 4 * N:(half + 1) * 4 * N],
                            adjq[half][:], op=OP.mult,
                        )
                        for mb in range(half * 4, half * 4 + 4):
                            for ch in range(2):
                                nc.tensor.matmul(
                                    at_ps[:, ch * 512:(ch + 1) * 512],
                                    aug[:, mb * 384 + hh * 48: mb * 384 + hh * 48 + 48],
                                    q_all[:, mb * N + ch * 512: mb * N + ch * 512 + 512],
                                    start=(mb == 0), stop=(mb == NB - 1),
                                )
                    nc.vector.tensor_copy(
                        stage_all[:, hh * N:(hh + 1) * N], at_ps[32:48, :]
                    )
                    lnz = work.tile([1, N], F32, tag="lnz")
                    nc.scalar.activation(lnz[:], at_ps[0:1, :], AF.Ln, bias=zbias[0:1])
                    zinv_h = work.tile([1, N], F16, tag="zinvh")
                    nc.scalar.activation(zinv_h[:], lnz[:], AF.Exp, scale=-1.0)
                    for ch in range(2):
                        nc.tensor.matmul(
                            zbc_ps[:, ch * 512:(ch + 1) * 512],
                            e16cat_t[0:1, hh * 128:(hh + 1) * 128],
                            zinv_h[0:1, ch * 512:(ch + 1) * 512],
                            start=(hh == 0), stop=(hh == H - 1),
                        )
                        nc.tensor.matmul(
                            asm_ps[:, ch * 512:(ch + 1) * 512],
                            sel_t[:, hh * 128:(hh + 1) * 128],
                            stage_all[:, hh * N + ch * 512: hh * N + ch * 512 + 512],
                            start=(hh == 0), stop=(hh == H - 1),
                        )

                stage_full = big.tile([128, N], F16)
                nc.scalar.activation(stage_full[:], asm_ps[:], AF.Copy)
                zbcf = big.tile([128, N], F16)
                nc.scalar.activation(zbcf[:], zbc_ps[:], AF.Copy)

            with tc.tile_pool(name="ps3", bufs=2, space="PSUM") as ps3:
                # ---- chunked epilogue: normalize+residual, LN1, FFN, LN2 ----
                C = 512

                def cs(t, c):
                    return t[:, c * C:(c + 1) * C]

                hh_t = big.tile([128, N], F16)
                x_res = big.tile([128, N], F16)

                def layernorm_T(x_in, g_col, b_col, out_tile, ps_pool, nm):
                    """Column-chunked transposed layernorm; J=ones/128 matmul
                    produces mean / mean-square directly as broadcast tiles."""
                    x2 = mid.tile([128, N], F16, tag=f"x2{nm}")
                    for c in range(N // C):
                        nc.vector.tensor_tensor(
                            cs(x2, c), cs(x_in, c), cs(x_in, c), op=OP.mult
                        )
                    for c in range(N // C):
                        mu_ps = ps_pool.tile([128, C], F32, tag="psb")
                        ssq_ps = ps_pool.tile([128, C], F32, tag="psb")
                        nc.tensor.matmul(mu_ps[:], jmat[:], cs(x_in, c),
                                         start=True, stop=True)
                        nc.tensor.matmul(ssq_ps[:], jmat[:], cs(x2, c),
                                         start=True, stop=True)
                        mu_bc = mid.tile([128, C], F16, tag=f"mbc{nm}{c}")
                        nc.scalar.activation(mu_bc[:], mu_ps[:], AF.Copy)
                        ssq_bc = mid.tile([128, C], F16, tag=f"sbc{nm}{c}")
                        nc.scalar.activation(ssq_bc[:], ssq_ps[:], AF.Copy)
                        mu2 = mid.tile([128, C], F16, tag=f"m2{nm}{c}")
                        nc.scalar.activation(mu2[:], mu_ps[:], AF.Square)
                        var = mid.tile([128, C], F16, tag=f"va{nm}{c}")
                        nc.vector.tensor_tensor(var[:], ssq_bc[:], mu2[:],
                                                op=OP.subtract)
                        lnv = mid.tile([128, C], F16, tag=f"lv{nm}{c}")
                        nc.scalar.activation(lnv[:], var[:], AF.Ln, bias=epsbias)
                        rstd = mid.tile([128, C], F16, tag=f"rs{nm}{c}")
                        nc.scalar.activation(rstd[:], lnv[:], AF.Exp, scale=-0.5)
                        t_ = mid.tile([128, C], F16, tag=f"lnt{nm}{c}")
                        nc.vector.tensor_tensor(t_[:], cs(x_in, c), mu_bc[:],
                                                op=OP.subtract)
                        xn = mid.tile([128, C], F16, tag=f"lnxn{nm}{c}")
                        nc.vector.tensor_tensor(xn[:], t_[:], rstd[:], op=OP.mult)
                        nc.vector.tensor_scalar(
                            cs(out_tile, c), xn[:], g_col[:], b_col[:],
                            op0=OP.mult, op1=OP.add,
                        )

                xc = big.tile([128, N], F16)
                y1s = big.tile([128, 2 * N], F16)
                y2b = big.tile([128, N], F16)
                z_res = big.tile([128, N], F16)
                outT_sb = big.tile([128, N], F16)

                for c in range(N // C):
                    nc.vector.tensor_tensor(cs(hh_t, c), cs(stage_full, c),
                                            cs(zbcf, c), op=OP.mult)
                    nc.vector.tensor_tensor(cs(x_res, c), cs(hh_t, c),
                                            cs(hT_t, c), op=OP.add)
                layernorm_T(x_res, g1_t, b1l_t, xc, ps3, "a")

                # FFN (chunked)
                for cb in range(2):
                    y1_ps = ps3.tile([128, N], F32, tag="ps3")
                    for c in range(N // C):
                        nc.tensor.matmul(
                            cs(y1_ps, c), w1_t[:, cb * 128:(cb + 1) * 128],
                            cs(xc, c), start=True, stop=True,
                        )
                        nc.scalar.activation(
                            y1s[:, cb * N + c * C: cb * N + (c + 1) * C],
                            cs(y1_ps, c), AF.Relu, bias=b1_t[:, cb:cb + 1],
                        )
                y2_ps = ps3.tile([128, N], F32, tag="ps3")
                for cb in range(2):
                    for c in range(N // C):
                        nc.tensor.matmul(
                            cs(y2_ps, c), w2_t[:, cb * 128:(cb + 1) * 128],
                            y1s[:, cb * N + c * C: cb * N + (c + 1) * C],
                            start=(cb == 0), stop=(cb == 1),
                        )
                for c in range(N // C):
                    nc.scalar.activation(cs(y2b, c), cs(y2_ps, c), AF.Identity,
                                         bias=b2_t)
                    nc.vector.tensor_tensor(cs(z_res, c), cs(y2b, c), cs(xc, c),
                                            op=OP.add)
                layernorm_T(z_res, g2_t, b2l_t, outT_sb, ps3, "b")
                for c in range(N // C):
                    nc.sync.dma_start(outT[:, c * C:(c + 1) * C],
                                      outT_sb[:, c * C:(c + 1) * C])

    nc.compile()
    return nc


def _host_prep(h, adj_mask, W, a, ln1_g, ln1_b, w1, b1, w2, b2, ln2_g, ln2_b):
    f16 = np.float16
    f32 = np.float32
    wcat = np.ascontiguousarray(
        np.transpose(np.asarray(W, f32), (1, 0, 2)).reshape(128, 128)
    ).astype(f16)
    a = np.asarray(a, f32)
    a_src, a_dst = a[:, :HD], a[:, HD:]
    Wf = np.asarray(W, f32)
    wa_dst = np.einsum("hid,hd->ih", Wf, a_dst).astype(f16)
    wa_src = np.einsum("hid,hd->ih", Wf, a_src)
    wasrep = np.repeat(wa_src[:, :, None], 128, axis=2).reshape(128, H * 128).astype(f16)
    sel = np.zeros((16, H * 128), f16)
    for hh in range(H):
        sel[np.arange(16), hh * 128 + hh * 16 + np.arange(16)] = 1.0
    e16cat = np.zeros((1, H * 128), f16)
    for hh in range(H):
        e16cat[0, hh * 128 + hh * 16: hh * 128 + (hh + 1) * 16] = 1.0
    w1c = np.asarray(w1, f32).astype(f16)
    w2c = np.ascontiguousarray(
        np.asarray(w2, f32).reshape(2, 128, 128).transpose(1, 0, 2).reshape(128, 256)
    ).astype(f16)
    augs = np.zeros((128, NB * 384), f16)
    augs[:, np.arange(NB * H) * 48] = 1.0  # ones columns
    wpackA = np.concatenate([wcat, wa_dst, wasrep], axis=1)
    wpackB = np.concatenate([w1c, w2c], axis=1)

    wpack32 = np.zeros((128, 9), f32)
    wpack32[:, 0:2] = np.asarray(b1, f32).reshape(2, 128).T
    wpack32[:, 2] = np.asarray(b2, f32)
    wpack32[:, 3] = np.asarray(ln1_g, f32)
    wpack32[:, 4] = np.asarray(ln1_b, f32)
    wpack32[:, 5] = np.asarray(ln2_g, f32)
    wpack32[:, 6] = np.asarray(ln2_b, f32)
    wpack32[:, 7] = 1e-4
    wpack32[:, 8] = EPS

    shared = dict(wpackA=wpackA, wpackB=wpackB, augpk=augs, wpack32=wpack32,
                  sel=sel, e16cat=e16cat)

    h = np.asarray(h, f32)
    adj = np.asarray(adj_mask)
    in_maps = []
    for b in range(B):
        hT = np.ascontiguousarray(h[b].T).astype(f16)
        adjT = np.ascontiguousarray(
            (adj[b] != 0).T.astype(f16).reshape(NB, 128, N).transpose(1, 0, 2).reshape(128, NB * N)
        )
        in_maps.append(dict(hT=hT, adjT=adjT, **shared))
    return in_maps


def kernel(**inputs):
    from concourse.bass_utils import run_bass_kernel_spmd

    if "nc" not in _CACHE:
        _CACHE["nc"] = _build_program()
    nc = _CACHE["nc"]

    in_maps = _host_prep(**inputs)
    res = run_bass_kernel_spmd(nc, in_maps, list(range(B)))
    out = np.empty((B, N, OUT_DIM), np.float32)
    for b in range(B):
        out[b] = res.results[b]["outT"].T
    return out


# revision 24
# speedup vs baseline: 1.0753x; 1.0753x over previous
"""MultiHeadGAT Trainium2 kernel: 8-core batch-parallel, transposed-layout pipeline.

Math: for scores e = lrelu(s_i[n] + s_j[m]), softmax numerator
  p = exp(lrelu(s_i+s_j)) = e^{0.2 s_i} * max(e^{0.8 s_i} * e^{s_j}, e^{0.2 s_j})
The e^{0.2 s_i} row factor cancels in softmax, so on-device we only compute
  q[m, n] = adjT[m, n] * max(Wbc[m, n] * u[m], v[m])
with Wbc = broadcast(e^{0.8 s_i}) (n-varying), u = e^{s_j}, v = e^{0.2 s_j}
(per-partition scalars), which is one fused tensor_scalar (mult+max) plus one
tensor_tensor (mask) per tile. Attention output and row-sum Z come from one
PE matmul with lhsT = [ones | pad | Wh_head]; normalization 1/Z = exp(-ln(Z)).
"""

import sys

sys.path.insert(0, "/opt/trn_rl_repo")

import numpy as np

B, N, IN_DIM, H, HD = 8, 1024, 128, 8, 16
OUT_DIM = H * HD
EPS = 1e-5
NB = N // 128  # 8 m-blocks

_CACHE = {}


def _patch_act_tables():
    # Force one activation table set for the whole kernel: every function we
    # use (Exp, Ln, Copy, Square, Relu, Identity) lives in
    # natural_log_exp_and_others; emptying the other sets makes Bacc's
    # table-load inserter emit exactly one ACT_TABLE_LOAD instead of
    # thrashing between exp/ln/small sets (~2.5us per reload).
    import concourse.bacc as bacc
    import concourse.hw_specs as hw_specs
    if getattr(bacc, "_act_tables_patched", False):
        return
    orig = hw_specs.get_activation_tables

    def patched(arch):
        t = dict(orig(arch))
        keep = "natural_log_exp_and_others"
        return {k: (v if k == keep else set()) for k, v in t.items()}

    bacc.get_activation_tables = patched
    bacc._act_tables_patched = True


def _build_program():
    import concourse.bacc as bacc
    import concourse.mybir as mybir
    import concourse.tile as tile

    _patch_act_tables()

    F16 = mybir.dt.float16
    F32 = mybir.dt.float32
    AF = mybir.ActivationFunctionType
    OP = mybir.AluOpType

    nc = bacc.Bacc("TRN2", target_bir_lowering=False, debug=False, num_devices=8)

    # ---- I/O ----
    hT = nc.dram_tensor("hT", [128, N], F16, kind="ExternalInput")
    adjT = nc.dram_tensor("adjT", [128, NB * N], F16, kind="ExternalInput")
    # critical pack: [wcat 128 | wadst 8 | wasrep 1024]
    wpackA = nc.dram_tensor("wpackA", [128, 1160], F16, kind="ExternalInput")
    # late pack: [w1 256 | w2 256]
    wpackB = nc.dram_tensor("wpackB", [128, 512], F16, kind="ExternalInput")
    augpk = nc.dram_tensor("augpk", [128, 3072], F16, kind="ExternalInput")
    # packed f32 cols: [b1c 2 | b2c 1 | g1 1 | b1l 1 | g2 1 | b2l 1 | zbias 1 | eps 1]
    wpack32 = nc.dram_tensor("wpack32", [128, 9], F32, kind="ExternalInput")
    sel = nc.dram_tensor("sel", [16, H * 128], F16, kind="ExternalInput")
    e16cat = nc.dram_tensor("e16cat", [1, H * 128], F16, kind="ExternalInput")
    outT = nc.dram_tensor("outT", [128, N], F16, kind="ExternalOutput")

    with tile.TileContext(nc) as tc:
        with (
            tc.tile_pool(name="const", bufs=1) as cpool,
            tc.tile_pool(name="big", bufs=1) as big,
            tc.tile_pool(name="work", bufs=2) as work,
            tc.tile_pool(name="mid", bufs=1) as mid,
            tc.tile_pool(name="rows", bufs=1) as rows,
        ):
            # ---- load everything ----
            # All on the SP (sync) HWDGE ring: FIFO order = priority order.
            hT_t = cpool.tile([128, N], F16)
            nc.sync.dma_start(hT_t[:], hT[:])
            wpA = cpool.tile([128, 1160], F16)
            nc.sync.dma_start(wpA[:], wpackA[:])
            adjq = [
                cpool.tile([128, 4 * N], F16, tag=f"adj{i}", name=f"adj{i}")
                for i in range(2)
            ]
            nc.sync.dma_start(adjq[0][:], adjT[:, 0:4 * N])
            augt = cpool.tile([128, 3072], F16)
            nc.sync.dma_start(augt[:], augpk[:])
            nc.sync.dma_start(adjq[1][:], adjT[:, 4 * N:8 * N])
            wpB = cpool.tile([128, 512], F16)
            nc.sync.dma_start(wpB[:], wpackB[:])
            wp32 = cpool.tile([128, 9], F32)
            nc.sync.dma_start(wp32[:], wpack32[:])
            sel_t = cpool.tile([16, H * 128], F16)
            nc.sync.dma_start(sel_t[:], sel[:])
            e16cat_t = cpool.tile([1, H * 128], F16)
            nc.sync.dma_start(e16cat_t[:], e16cat[:])

            wcat_t = wpA[:, 0:128]
            wadst_t = wpA[:, 128:136]
            wasrep_t = wpA[:, 136:1160]
            w1_t = wpB[:, 0:256]
            w2_t = wpB[:, 256:512]
            aug = augt[:]
            b1_t = wp32[:, 0:2]
            b2_t = wp32[:, 2:3]
            g1_t = wp32[:, 3:4]
            b1l_t = wp32[:, 4:5]
            g2_t = wp32[:, 5:6]
            b2l_t = wp32[:, 6:7]
            zbias = wp32[:, 7:8]
            epsbias = wp32[:, 8:9]

            onescol = cpool.tile([128, 1], F16)
            nc.vector.memset(onescol[:], 1.0)
            jmat = cpool.tile([128, 128], F16)
            nc.vector.memset(jmat[:], 1.0 / 128)
            onesrow = cpool.tile([1, 128], F32)
            nc.vector.memset(onesrow[:], 1.0)

            # ---- phase 1: s-cols(u,v), Wbc, Wh_nat->aug ----
            u_t = [big.tile([128, H], F32, tag=f"u{i}", name=f"u{i}") for i in range(NB)]
            v_t = [big.tile([128, H], F32, tag=f"v{i}", name=f"v{i}") for i in range(NB)]
            wbc = [big.tile([128, N], F16, tag=f"wbc{i}", name=f"wbc{i}") for i in range(H)]
            aug4w = aug.rearrange("p (m h c) -> p m h c", m=NB, h=H, c=48)

            with tc.tile_pool(name="ps1", bufs=3, space="PSUM") as ps1:
                for mb in range(NB):
                    sc_ps = ps1.tile([128, H], F32, tag="ps1")
                    nc.tensor.matmul(
                        sc_ps[:], hT_t[:, mb * 128:(mb + 1) * 128], wadst_t,
                        start=True, stop=True,
                    )
                    nc.scalar.activation(u_t[mb][:], sc_ps[:], AF.Exp, scale=1.0)
                    nc.scalar.activation(v_t[mb][:], sc_ps[:], AF.Exp, scale=0.2)
                    if mb == 0:
                        wb_ps = ps1.tile([128, N], F32, tag="ps1")
                        for ch in range(2):
                            nc.tensor.matmul(
                                wb_ps[:, ch * 512:(ch + 1) * 512],
                                wasrep_t[:, 0:128],
                                hT_t[:, ch * 512:(ch + 1) * 512],
                                start=True, stop=True,
                            )
                        nc.scalar.activation(wbc[0][:], wb_ps[:], AF.Exp, scale=0.8)
                for hh in range(1, H):
                    wb_ps = ps1.tile([128, N], F32, tag="ps1")
                    for ch in range(2):
                        nc.tensor.matmul(
                            wb_ps[:, ch * 512:(ch + 1) * 512],
                            wasrep_t[:, hh * 128:(hh + 1) * 128],
                            hT_t[:, ch * 512:(ch + 1) * 512],
                            start=True, stop=True,
                        )
                    nc.scalar.activation(wbc[hh][:], wb_ps[:], AF.Exp, scale=0.8)
                for mb in range(NB):
                    wn_ps = ps1.tile([128, 128], F32, tag="ps1")
                    nc.tensor.matmul(
                        wn_ps[:], hT_t[:, mb * 128:(mb + 1) * 128], wcat_t,
                        start=True, stop=True,
                    )
                    wn4 = wn_ps[:].rearrange("p (h d) -> p h d", h=H, d=16)
                    nc.scalar.activation(aug4w[:, mb, :, 32:48], wn4[:], AF.Copy)

            # ---- phase 2: attention ----
            stage_all = big.tile([16, H * N], F16)
            with (
                tc.tile_pool(name="ps48", bufs=2, space="PSUM") as ps48,
                tc.tile_pool(name="psacc", bufs=1, space="PSUM") as psacc,
                tc.tile_pool(name="psz", bufs=1, space="PSUM") as psz,
            ):
                asm_ps = psacc.tile([128, N], F32)
                zbc_ps = psz.tile([128, N], F32)
                for hh in range(H):
                    q_all = work.tile([128, NB * N], F16, tag="q")
                    t1 = work.tile([128, NB * N], F16, tag="t1")
                    at_ps = ps48.tile([48, N], F32)
                    for half in range(2):
                        for mb in range(half * 4, half * 4 + 4):
                            nc.vector.tensor_scalar(
                                t1[:, mb * N:(mb + 1) * N],
                                wbc[hh][:],
                                u_t[mb][:, hh: hh + 1],
                                v_t[mb][:, hh: hh + 1],
                                op0=OP.mult, op1=OP.max,
                            )
                        nc.vector.tensor_tensor(
                            q_all[:, half * 4 * N:(half + 1) * 4 * N],
                            t1[:, half * 4 * N:(half + 1) * 4 * N],
                            adjq[half][:], op=OP.mult,
                        )
                        for mb in range(half * 4, half * 4 + 4):
                            for ch in range(2):
                                nc.tensor.matmul(
                                    at_ps[:, ch * 512:(ch + 1) * 512],
                                    aug[:, mb * 384 + hh * 48: mb * 384 + hh * 48 + 48],
                                    q_all[:, mb * N + ch * 512: mb * N + ch * 512 + 512],
                                    start=(mb == 0), stop=(mb == NB - 1),
                                )
                    nc.scalar.activation(
                        stage_all[:, hh * N:(hh + 1) * N], at_ps[32:48, :], AF.Copy
                    )
                    lnz = work.tile([1, N], F32, tag="lnz")
                    nc.scalar.activation(lnz[:], at_ps[0:1, :], AF.Ln, bias=zbias[0:1])
                    zinv_h = work.tile([1, N], F16, tag="zinvh")
                    nc.scalar.activation(zinv_h[:], lnz[:], AF.Exp, scale=-1.0)
                    for ch in range(2):
                        nc.tensor.matmul(
                            zbc_ps[:, ch * 512:(ch + 1) * 512],
                            e16cat_t[0:1, hh * 128:(hh + 1) * 128],
                            zinv_h[0:1, ch * 512:(ch + 1) * 512],
                            start=(hh == 0), stop=(hh == H - 1),
                        )
                        nc.tensor.matmul(
                            asm_ps[:, ch * 512:(ch + 1) * 512],
                            sel_t[:, hh * 128:(hh + 1) * 128],
                            stage_all[:, hh * N + ch * 512: hh * N + ch * 512 + 512],
                            start=(hh == 0), stop=(hh == H - 1),
                        )

                stage_full = big.tile([128, N], F16)
                nc.scalar.activation(stage_full[:], asm_ps[:], AF.Copy)
                zbcf = big.tile([128, N], F16)
                nc.scalar.activation(zbcf[:], zbc_ps[:], AF.Copy)

            with tc.tile_pool(name="ps3", bufs=2, space="PSUM") as ps3:
                # ---- chunked epilogue: normalize+residual, LN1, FFN, LN2 ----
                C = 512

                def cs(t, c):
                    return t[:, c * C:(c + 1) * C]

                hh_t = big.tile([128, N], F16)
                x_res = big.tile([128, N], F16)

                def layernorm_T(x_in, g_col, b_col, out_tile, ps_pool, nm):
                    """Column-chunked transposed layernorm; J=ones/128 matmul
                    produces mean / mean-square directly as broadcast tiles."""
                    x2 = mid.tile([128, N], F16, tag=f"x2{nm}")
                    for c in range(N // C):
                        nc.vector.tensor_tensor(
                            cs(x2, c), cs(x_in, c), cs(x_in, c), op=OP.mult
                        )
                    for c in range(N // C):
                        mu_ps = ps_pool.tile([128, C], F32, tag="psb")
                        ssq_ps = ps_pool.tile([128, C], F32, tag="psb")
                        nc.tensor.matmul(mu_ps[:], jmat[:], cs(x_in, c),
                                         start=True, stop=True)
                        nc.tensor.matmul(ssq_ps[:], jmat[:], cs(x2, c),
                                         start=True, stop=True)
                        mu_bc = mid.tile([128, C], F16, tag=f"mbc{nm}{c}")
                        nc.scalar.activation(mu_bc[:], mu_ps[:], AF.Copy)
                        ssq_bc = mid.tile([128, C], F16, tag=f"sbc{nm}{c}")
                        nc.scalar.activation(ssq_bc[:], ssq_ps[:], AF.Copy)
                        mu2 = mid.tile([128, C], F16, tag=f"m2{nm}{c}")
                        nc.scalar.activation(mu2[:], mu_ps[:], AF.Square)
                        var = mid.tile([128, C], F16, tag=f"va{nm}{c}")
                        nc.vector.tensor_tensor(var[:], ssq_bc[:], mu2[:],
                                                op=OP.subtract)
                        lnv = mid.tile([128, C], F16, tag=f"lv{nm}{c}")
                        nc.scalar.activation(lnv[:], var[:], AF.Ln, bias=epsbias)
                        rstd = mid.tile([128, C], F16, tag=f"rs{nm}{c}")
                        nc.scalar.activation(rstd[:], lnv[:], AF.Exp, scale=-0.5)
                        t_ = mid.tile([128, C], F16, tag=f"lnt{nm}{c}")
                        nc.vector.tensor_tensor(t_[:], cs(x_in, c), mu_bc[:],
                                                op=OP.subtract)
                        xn = mid.tile([128, C], F16, tag=f"lnxn{nm}{c}")
                        nc.vector.tensor_tensor(xn[:], t_[:], rstd[:], op=OP.mult)
                        nc.vector.tensor_scalar(
                            cs(out_tile, c), xn[:], g_col[:], b_col[:],
                            op0=OP.mult, op1=OP.add,
                        )

                xc = big.tile([128, N], F16)
                y1s = big.tile([128, 2 * N], F16)
                y2b = big.tile([128, N], F16)
                z_res = big.tile([128, N], F16)
                outT_sb = big.tile([128, N], F16)

                for c in range(N // C):
                    nc.vector.tensor_tensor(cs(hh_t, c), cs(stage_full, c),
                                            cs(zbcf, c), op=OP.mult)
                    nc.vector.tensor_tensor(cs(x_res, c), cs(hh_t, c),
                                            cs(hT_t, c), op=OP.add)
                layernorm_T(x_res, g1_t, b1l_t, xc, ps3, "a")

                # FFN (chunked)
                for cb in range(2):
                    y1_ps = ps3.tile([128, N], F32, tag="ps3")
                    for c in range(N // C):
                        nc.tensor.matmul(
                            cs(y1_ps, c), w1_t[:, cb * 128:(cb + 1) * 128],
                            cs(xc, c), start=True, stop=True,
                        )
                        nc.scalar.activation(
                            y1s[:, cb * N + c * C: cb * N + (c + 1) * C],
                            cs(y1_ps, c), AF.Relu, bias=b1_t[:, cb:cb + 1],
                        )
                y2_ps = ps3.tile([128, N], F32, tag="ps3")
                for cb in range(2):
                    for c in range(N // C):
                        nc.tensor.matmul(
                            cs(y2_ps, c), w2_t[:, cb * 128:(cb + 1) * 128],
                            y1s[:, cb * N + c * C: cb * N + (c + 1) * C],
                            start=(cb == 0), stop=(cb == 1),
                        )
                for c in range(N // C):
                    nc.scalar.activation(cs(y2b, c), cs(y2_ps, c), AF.Identity,
                                         bias=b2_t)
                    nc.vector.tensor_tensor(cs(z_res, c), cs(y2b, c), cs(xc, c),
                                            op=OP.add)
                layernorm_T(z_res, g2_t, b2l_t, outT_sb, ps3, "b")
                for c in range(N // C):
                    nc.sync.dma_start(outT[:, c * C:(c + 1) * C],
                                      outT_sb[:, c * C:(c + 1) * C])

    nc.compile()
    return nc


def _host_prep(h, adj_mask, W, a, ln1_g, ln1_b, w1, b1, w2, b2, ln2_g, ln2_b):
    f16 = np.float16
    f32 = np.float32
    wcat = np.ascontiguousarray(
        np.transpose(np.asarray(W, f32), (1, 0, 2)).reshape(128, 128)
    ).astype(f16)
    a = np.asarray(a, f32)
    a_src, a_dst = a[:, :HD], a[:, HD:]
    Wf = np.asarray(W, f32)
    wa_dst = np.einsum("hid,hd->ih", Wf, a_dst).astype(f16)
    wa_src = np.einsum("hid,hd->ih", Wf, a_src)
    wasrep = np.repeat(wa_src[:, :, None], 128, axis=2).reshape(128, H * 128).astype(f16)
    sel = np.zeros((16, H * 128), f16)
    for hh in range(H):
        sel[np.arange(16), hh * 128 + hh * 16 + np.arange(16)] = 1.0
    e16cat = np.zeros((1, H * 128), f16)
    for hh in range(H):
        e16cat[0, hh * 128 + hh * 16: hh * 128 + (hh + 1) * 16] = 1.0
    w1c = np.asarray(w1, f32).astype(f16)
    w2c = np.ascontiguousarray(
        np.asarray(w2, f32).reshape(2, 128, 128).transpose(1, 0, 2).reshape(128, 256)
    ).astype(f16)
    augs = np.zeros((128, NB * 384), f16)
    augs[:, np.arange(NB * H) * 48] = 1.0  # ones columns
    wpackA = np.concatenate([wcat, wa_dst, wasrep], axis=1)
    wpackB = np.concatenate([w1c, w2c], axis=1)

    wpack32 = np.zeros((128, 9), f32)
    wpack32[:, 0:2] = np.asarray(b1, f32).reshape(2, 128).T
    wpack32[:, 2] = np.asarray(b2, f32)
    wpack32[:, 3] = np.asarray(ln1_g, f32)
    wpack32[:, 4] = np.asarray(ln1_b, f32)
    wpack32[:, 5] = np.asarray(ln2_g, f32)
    wpack32[:, 6] = np.asarray(ln2_b, f32)
    wpack32[:, 7] = 1e-4
    wpack32[:, 8] = EPS

    shared = dict(wpackA=wpackA, wpackB=wpackB, augpk=augs, wpack32=wpack32,
                  sel=sel, e16cat=e16cat)

    h = np.asarray(h, f32)
    adj = np.asarray(adj_mask)
    in_maps = []
    for b in range(B):
        hT = np.ascontiguousarray(h[b].T).astype(f16)
        adjT = np.ascontiguousarray(
            (adj[b] != 0).T.astype(f16).reshape(NB, 128, N).transpose(1, 0, 2).reshape(128, NB * N)
        )
        in_maps.append(dict(hT=hT, adjT=adjT, **shared))
    return in_maps


def kernel(**inputs):
    from concourse.bass_utils import run_bass_kernel_spmd

    if "nc" not in _CACHE:
        _CACHE["nc"] = _build_program()
    nc = _CACHE["nc"]

    in_maps = _host_prep(**inputs)
    res = run_bass_kernel_spmd(nc, in_maps, list(range(B)))
    out = np.empty((B, N, OUT_DIM), np.float32)
    for b in range(B):
        out[b] = res.results[b]["outT"].T
    return out


# revision 25
# speedup vs baseline: 1.1195x; 1.0411x over previous
"""MultiHeadGAT Trainium2 kernel: 8-core batch-parallel, transposed-layout pipeline.

Math: for scores e = lrelu(s_i[n] + s_j[m]), softmax numerator
  p = exp(lrelu(s_i+s_j)) = e^{0.2 s_i} * max(e^{0.8 s_i} * e^{s_j}, e^{0.2 s_j})
The e^{0.2 s_i} row factor cancels in softmax, so on-device we only compute
  q[m, n] = adjT[m, n] * max(Wbc[m, n] * u[m], v[m])
with Wbc = broadcast(e^{0.8 s_i}) (n-varying), u = e^{s_j}, v = e^{0.2 s_j}
(per-partition scalars), which is one fused tensor_scalar (mult+max) plus one
tensor_tensor (mask) per tile. Attention output and row-sum Z come from one
PE matmul with lhsT = [ones | pad | Wh_head]; normalization 1/Z = exp(-ln(Z)).
"""

import sys

sys.path.insert(0, "/opt/trn_rl_repo")

import numpy as np

B, N, IN_DIM, H, HD = 8, 1024, 128, 8, 16
OUT_DIM = H * HD
EPS = 1e-5
NB = N // 128  # 8 m-blocks

_CACHE = {}


def _patch_act_tables():
    # Force one activation table set for the whole kernel: every function we
    # use (Exp, Ln, Copy, Square, Relu, Identity) lives in
    # natural_log_exp_and_others; emptying the other sets makes Bacc's
    # table-load inserter emit exactly one ACT_TABLE_LOAD instead of
    # thrashing between exp/ln/small sets (~2.5us per reload).
    import concourse.bacc as bacc
    import concourse.hw_specs as hw_specs
    if getattr(bacc, "_act_tables_patched", False):
        return
    orig = hw_specs.get_activation_tables

    def patched(arch):
        t = dict(orig(arch))
        keep = "natural_log_exp_and_others"
        return {k: (v if k == keep else set()) for k, v in t.items()}

    bacc.get_activation_tables = patched
    bacc._act_tables_patched = True


def _build_program():
    import concourse.bacc as bacc
    import concourse.mybir as mybir
    import concourse.tile as tile

    _patch_act_tables()

    F16 = mybir.dt.float16
    F32 = mybir.dt.float32
    AF = mybir.ActivationFunctionType
    OP = mybir.AluOpType

    nc = bacc.Bacc("TRN2", target_bir_lowering=False, debug=False, num_devices=8)

    # ---- I/O ----
    hT = nc.dram_tensor("hT", [128, N], F16, kind="ExternalInput")
    adjT = nc.dram_tensor("adjT", [128, NB * N], F16, kind="ExternalInput")
    # critical pack: [wcat 128 | wadst 8 | wasrep 1024]
    wpackA = nc.dram_tensor("wpackA", [128, 1160], F16, kind="ExternalInput")
    # late pack: [w1 256 | w2 256]
    wpackB = nc.dram_tensor("wpackB", [128, 512], F16, kind="ExternalInput")
    augpk = nc.dram_tensor("augpk", [128, 3072], F16, kind="ExternalInput")
    # packed f32 cols: [b1c 2 | b2c 1 | g1 1 | b1l 1 | g2 1 | b2l 1 | zbias 1 | eps 1]
    wpack32 = nc.dram_tensor("wpack32", [128, 9], F32, kind="ExternalInput")
    sel = nc.dram_tensor("sel", [16, H * 128], F16, kind="ExternalInput")
    e16cat = nc.dram_tensor("e16cat", [1, H * 128], F16, kind="ExternalInput")
    outT = nc.dram_tensor("outT", [128, N], F16, kind="ExternalOutput")

    with tile.TileContext(nc) as tc:
        with (
            tc.tile_pool(name="const", bufs=1) as cpool,
            tc.tile_pool(name="big", bufs=1) as big,
            tc.tile_pool(name="work", bufs=2) as work,
            tc.tile_pool(name="mid", bufs=1) as mid,
            tc.tile_pool(name="rows", bufs=1) as rows,
        ):
            # ---- load everything ----
            # All on the SP (sync) HWDGE ring: FIFO order = priority order.
            hT_t = cpool.tile([128, N], F16)
            nc.sync.dma_start(hT_t[:], hT[:])
            wpA = cpool.tile([128, 1160], F16)
            nc.sync.dma_start(wpA[:], wpackA[:])
            adjq = [
                cpool.tile([128, 4 * N], F16, tag=f"adj{i}", name=f"adj{i}")
                for i in range(2)
            ]
            nc.sync.dma_start(adjq[0][:], adjT[:, 0:4 * N])
            augt = cpool.tile([128, 3072], F16)
            nc.sync.dma_start(augt[:], augpk[:])
            nc.sync.dma_start(adjq[1][:], adjT[:, 4 * N:8 * N])
            wpB = cpool.tile([128, 512], F16)
            nc.sync.dma_start(wpB[:], wpackB[:])
            wp32 = cpool.tile([128, 9], F32)
            nc.sync.dma_start(wp32[:], wpack32[:])
            sel_t = cpool.tile([16, H * 128], F16)
            nc.sync.dma_start(sel_t[:], sel[:])
            e16cat_t = cpool.tile([1, H * 128], F16)
            nc.sync.dma_start(e16cat_t[:], e16cat[:])

            wcat_t = wpA[:, 0:128]
            wadst_t = wpA[:, 128:136]
            wasrep_t = wpA[:, 136:1160]
            w1_t = wpB[:, 0:256]
            w2_t = wpB[:, 256:512]
            aug = augt[:]
            b1_t = wp32[:, 0:2]
            b2_t = wp32[:, 2:3]
            g1_t = wp32[:, 3:4]
            b1l_t = wp32[:, 4:5]
            g2_t = wp32[:, 5:6]
            b2l_t = wp32[:, 6:7]
            zbias = wp32[:, 7:8]
            epsbias = wp32[:, 8:9]

            onescol = cpool.tile([128, 1], F16)
            nc.vector.memset(onescol[:], 1.0)
            jmat = cpool.tile([128, 128], F16)
            nc.vector.memset(jmat[:], 1.0 / 128)
            onesrow = cpool.tile([1, 128], F32)
            nc.vector.memset(onesrow[:], 1.0)

            # ---- phase 1: s-cols(u,v), Wbc, Wh_nat->aug ----
            u_t = [big.tile([128, H], F32, tag=f"u{i}", name=f"u{i}") for i in range(NB)]
            v_t = [big.tile([128, H], F32, tag=f"v{i}", name=f"v{i}") for i in range(NB)]
            wbc = [big.tile([128, N], F16, tag=f"wbc{i}", name=f"wbc{i}") for i in range(H)]
            aug4w = aug.rearrange("p (m h c) -> p m h c", m=NB, h=H, c=48)

            with tc.tile_pool(name="ps1", bufs=3, space="PSUM") as ps1:
                for mb in range(NB):
                    sc_ps = ps1.tile([128, H], F32, tag="ps1")
                    nc.tensor.matmul(
                        sc_ps[:], hT_t[:, mb * 128:(mb + 1) * 128], wadst_t,
                        start=True, stop=True,
                    )
                    nc.scalar.activation(u_t[mb][:], sc_ps[:], AF.Exp, scale=1.0)
                    nc.scalar.activation(v_t[mb][:], sc_ps[:], AF.Exp, scale=0.2)
                    if mb == 0:
                        wb_ps = ps1.tile([128, N], F32, tag="ps1")
                        for ch in range(2):
                            nc.tensor.matmul(
                                wb_ps[:, ch * 512:(ch + 1) * 512],
                                wasrep_t[:, 0:128],
                                hT_t[:, ch * 512:(ch + 1) * 512],
                                start=True, stop=True,
                            )
                        nc.scalar.activation(wbc[0][:], wb_ps[:], AF.Exp, scale=0.8)
                for hh in range(1, H):
                    wb_ps = ps1.tile([128, N], F32, tag="ps1")
                    for ch in range(2):
                        nc.tensor.matmul(
                            wb_ps[:, ch * 512:(ch + 1) * 512],
                            wasrep_t[:, hh * 128:(hh + 1) * 128],
                            hT_t[:, ch * 512:(ch + 1) * 512],
                            start=True, stop=True,
                        )
                    nc.scalar.activation(wbc[hh][:], wb_ps[:], AF.Exp, scale=0.8)
                for mb in range(NB):
                    wn_ps = ps1.tile([128, 128], F32, tag="ps1")
                    nc.tensor.matmul(
                        wn_ps[:], hT_t[:, mb * 128:(mb + 1) * 128], wcat_t,
                        start=True, stop=True,
                    )
                    wn4 = wn_ps[:].rearrange("p (h d) -> p h d", h=H, d=16)
                    nc.scalar.activation(aug4w[:, mb, :, 32:48], wn4[:], AF.Copy)

            # ---- phase 2: attention ----
            stage_all = big.tile([16, H * N], F16)
            with (
                tc.tile_pool(name="ps48", bufs=2, space="PSUM") as ps48,
                tc.tile_pool(name="psacc", bufs=1, space="PSUM") as psacc,
                tc.tile_pool(name="psz", bufs=1, space="PSUM") as psz,
            ):
                asm_ps = psacc.tile([128, N], F32)
                zbc_ps = psz.tile([128, N], F32)
                for hh in range(H):
                    q_all = work.tile([128, NB * N], F16, tag="q")
                    t1 = work.tile([128, NB * N], F16, tag="t1")
                    at_ps = ps48.tile([48, N], F32)
                    for half in range(2):
                        for mb in range(half * 4, half * 4 + 4):
                            nc.vector.tensor_scalar(
                                t1[:, mb * N:(mb + 1) * N],
                                wbc[hh][:],
                                u_t[mb][:, hh: hh + 1],
                                v_t[mb][:, hh: hh + 1],
                                op0=OP.mult, op1=OP.max,
                            )
                        nc.vector.tensor_tensor(
                            q_all[:, half * 4 * N:(half + 1) * 4 * N],
                            t1[:, half * 4 * N:(half + 1) * 4 * N],
                            adjq[half][:], op=OP.mult,
                        )
                        for mb in range(half * 4, half * 4 + 4):
                            for ch in range(2):
                                nc.tensor.matmul(
                                    at_ps[:, ch * 512:(ch + 1) * 512],
                                    aug[:, mb * 384 + hh * 48: mb * 384 + hh * 48 + 48],
                                    q_all[:, mb * N + ch * 512: mb * N + ch * 512 + 512],
                                    start=(mb == 0), stop=(mb == NB - 1),
                                )
                    nc.scalar.activation(
                        stage_all[:, hh * N:(hh + 1) * N], at_ps[32:48, :], AF.Copy
                    )
                    lnz = work.tile([1, N], F32, tag="lnz")
                    nc.scalar.activation(lnz[:], at_ps[0:1, :], AF.Ln, bias=zbias[0:1])
                    zinv_h = work.tile([1, N], F16, tag="zinvh")
                    nc.scalar.activation(zinv_h[:], lnz[:], AF.Exp, scale=-1.0)
                    for ch in range(2):
                        nc.tensor.matmul(
                            zbc_ps[:, ch * 512:(ch + 1) * 512],
                            e16cat_t[0:1, hh * 128:(hh + 1) * 128],
                            zinv_h[0:1, ch * 512:(ch + 1) * 512],
                            start=(hh == 0), stop=(hh == H - 1),
                        )
                        nc.tensor.matmul(
                            asm_ps[:, ch * 512:(ch + 1) * 512],
                            sel_t[:, hh * 128:(hh + 1) * 128],
                            stage_all[:, hh * N + ch * 512: hh * N + ch * 512 + 512],
                            start=(hh == 0), stop=(hh == H - 1),
                        )

                stage_full = big.tile([128, N], F16)
                nc.scalar.activation(stage_full[:], asm_ps[:], AF.Copy)
                zbcf = big.tile([128, N], F16)
                nc.scalar.activation(zbcf[:], zbc_ps[:], AF.Copy)

            with tc.tile_pool(name="ps3", bufs=2, space="PSUM") as ps3:
                # ---- chunked epilogue: normalize+residual, LN1, FFN, LN2 ----
                C = 512

                def cs(t, c):
                    return t[:, c * C:(c + 1) * C]

                hh_t = big.tile([128, N], F16)
                x_res = big.tile([128, N], F16)

                def layernorm_T(x_in, g_col, b_col, out_tile, ps_pool, nm):
                    """Column-chunked transposed layernorm; J=ones/128 matmul
                    produces mean / mean-square directly as broadcast tiles."""
                    x2 = mid.tile([128, N], F16, tag=f"x2{nm}")
                    for c in range(N // C):
                        nc.vector.tensor_tensor(
                            cs(x2, c), cs(x_in, c), cs(x_in, c), op=OP.mult
                        )
                    for c in range(N // C):
                        mu_ps = ps_pool.tile([128, C], F32, tag="psb")
                        ssq_ps = ps_pool.tile([128, C], F32, tag="psb")
                        nc.tensor.matmul(mu_ps[:], jmat[:], cs(x_in, c),
                                         start=True, stop=True)
                        nc.tensor.matmul(ssq_ps[:], jmat[:], cs(x2, c),
                                         start=True, stop=True)
                        mu_bc = mid.tile([128, C], F16, tag=f"mbc{nm}{c}")
                        nc.scalar.activation(mu_bc[:], mu_ps[:], AF.Copy)
                        ssq_bc = mid.tile([128, C], F16, tag=f"sbc{nm}{c}")
                        nc.scalar.activation(ssq_bc[:], ssq_ps[:], AF.Copy)
                        mu2 = mid.tile([128, C], F16, tag=f"m2{nm}{c}")
                        nc.vector.tensor_tensor(mu2[:], mu_bc[:], mu_bc[:], op=OP.mult)
                        var = mid.tile([128, C], F16, tag=f"va{nm}{c}")
                        nc.vector.tensor_tensor(var[:], ssq_bc[:], mu2[:],
                                                op=OP.subtract)
                        lnv = mid.tile([128, C], F16, tag=f"lv{nm}{c}")
                        nc.scalar.activation(lnv[:], var[:], AF.Ln, bias=epsbias)
                        rstd = mid.tile([128, C], F16, tag=f"rs{nm}{c}")
                        nc.scalar.activation(rstd[:], lnv[:], AF.Exp, scale=-0.5)
                        t_ = mid.tile([128, C], F16, tag=f"lnt{nm}{c}")
                        nc.vector.tensor_tensor(t_[:], cs(x_in, c), mu_bc[:],
                                                op=OP.subtract)
                        xn = mid.tile([128, C], F16, tag=f"lnxn{nm}{c}")
                        nc.vector.tensor_tensor(xn[:], t_[:], rstd[:], op=OP.mult)
                        nc.vector.tensor_scalar(
                            cs(out_tile, c), xn[:], g_col[:], b_col[:],
                            op0=OP.mult, op1=OP.add,
                        )

                xc = big.tile([128, N], F16)
                y1s = big.tile([128, 2 * N], F16)
                y2b = big.tile([128, N], F16)
                z_res = big.tile([128, N], F16)
                outT_sb = big.tile([128, N], F16)

                for c in range(N // C):
                    nc.vector.tensor_tensor(cs(hh_t, c), cs(stage_full, c),
                                            cs(zbcf, c), op=OP.mult)
                    nc.vector.tensor_tensor(cs(x_res, c), cs(hh_t, c),
                                            cs(hT_t, c), op=OP.add)
                layernorm_T(x_res, g1_t, b1l_t, xc, ps3, "a")

                # FFN (chunked)
                for cb in range(2):
                    y1_ps = ps3.tile([128, N], F32, tag="ps3")
                    for c in range(N // C):
                        nc.tensor.matmul(
                            cs(y1_ps, c), w1_t[:, cb * 128:(cb + 1) * 128],
                            cs(xc, c), start=True, stop=True,
                        )
                        nc.scalar.activation(
                            y1s[:, cb * N + c * C: cb * N + (c + 1) * C],
                            cs(y1_ps, c), AF.Relu, bias=b1_t[:, cb:cb + 1],
                        )
                y2_ps = ps3.tile([128, N], F32, tag="ps3")
                for cb in range(2):
                    for c in range(N // C):
                        nc.tensor.matmul(
                            cs(y2_ps, c), w2_t[:, cb * 128:(cb + 1) * 128],
                            y1s[:, cb * N + c * C: cb * N + (c + 1) * C],
                            start=(cb == 0), stop=(cb == 1),
                        )
                for c in range(N // C):
                    nc.scalar.activation(cs(y2b, c), cs(y2_ps, c), AF.Identity,
                                         bias=b2_t)
                    nc.vector.tensor_tensor(cs(z_res, c), cs(y2b, c), cs(xc, c),
                                            op=OP.add)
                layernorm_T(z_res, g2_t, b2l_t, outT_sb, ps3, "b")
                for c in range(N // C):
                    nc.sync.dma_start(outT[:, c * C:(c + 1) * C],
                                      outT_sb[:, c * C:(c + 1) * C])

    nc.compile()
    return nc


def _host_prep(h, adj_mask, W, a, ln1_g, ln1_b, w1, b1, w2, b2, ln2_g, ln2_b):
    f16 = np.float16
    f32 = np.float32
    wcat = np.ascontiguousarray(
        np.transpose(np.asarray(W, f32), (1, 0, 2)).reshape(128, 128)
    ).astype(f16)
    a = np.asarray(a, f32)
    a_src, a_dst = a[:, :HD], a[:, HD:]
    Wf = np.asarray(W, f32)
    wa_dst = np.einsum("hid,hd->ih", Wf, a_dst).astype(f16)
    wa_src = np.einsum("hid,hd->ih", Wf, a_src)
    wasrep = np.repeat(wa_src[:, :, None], 128, axis=2).reshape(128, H * 128).astype(f16)
    sel = np.zeros((16, H * 128), f16)
    for hh in range(H):
        sel[np.arange(16), hh * 128 + hh * 16 + np.arange(16)] = 1.0
    e16cat = np.zeros((1, H * 128), f16)
    for hh in range(H):
        e16cat[0, hh * 128 + hh * 16: hh * 128 + (hh + 1) * 16] = 1.0
    w1c = np.asarray(w1, f32).astype(f16)
    w2c = np.ascontiguousarray(
        np.asarray(w2, f32).reshape(2, 128, 128).transpose(1, 0, 2).reshape(128, 256)
    ).astype(f16)
    augs = np.zeros((128, NB * 384), f16)
    augs[:, np.arange(NB * H) * 48] = 1.0  # ones columns
    wpackA = np.concatenate([wcat, wa_dst, wasrep], axis=1)
    wpackB = np.concatenate([w1c, w2c], axis=1)

    wpack32 = np.zeros((128, 9), f32)
    wpack32[:, 0:2] = np.asarray(b1, f32).reshape(2, 128).T
    wpack32[:, 2] = np.asarray(b2, f32)
    wpack32[:, 3] = np.asarray(ln1_g, f32)
    wpack32[:, 4] = np.asarray(ln1_b, f32)
    wpack32[:, 5] = np.asarray(ln2_g, f32)
    wpack32[:, 6] = np.asarray(ln2_b, f32)
    wpack32[:, 7] = 1e-4
    wpack32[:, 8] = EPS

    shared = dict(wpackA=wpackA, wpackB=wpackB, augpk=augs, wpack32=wpack32,
                  sel=sel, e16cat=e16cat)

    h = np.asarray(h, f32)
    adj = np.asarray(adj_mask)
    in_maps = []
    for b in range(B):
        hT = np.ascontiguousarray(h[b].T).astype(f16)
        adjT = np.ascontiguousarray(
            (adj[b] != 0).T.astype(f16).reshape(NB, 128, N).transpose(1, 0, 2).reshape(128, NB * N)
        )
        in_maps.append(dict(hT=hT, adjT=adjT, **shared))
    return in_maps


def kernel(**inputs):
    from concourse.bass_utils import run_bass_kernel_spmd

    if "nc" not in _CACHE:
        _CACHE["nc"] = _build_program()
    nc = _CACHE["nc"]

    in_maps = _host_prep(**inputs)
    res = run_bass_kernel_spmd(nc, in_maps, list(range(B)))
    out = np.empty((B, N, OUT_DIM), np.float32)
    for b in range(B):
        out[b] = res.results[b]["outT"].T
    return out
